# revision 1
# baseline (speedup 1.0000x reference)
"""Trainium2 Bass kernel for nn_BlockTransformer (Octo-style block-sparse transformer).

Strategy: data-parallel over batch (B=8 -> 1 element per NeuronCore), weights
replicated. Residual stream kept transposed (D on partitions) to avoid all
on-device transposes. Tokens reordered to [prefix|pad, obs t0..t9 (128-aligned),
readouts|pad] = 1536 padded tokens so the block-sparse attention mask becomes
128-aligned; per-key mask folded into the softmax-exp bias (per-partition),
readout causality handled by small memsets + one static 0/1 multiplier tile.
Matmuls in bf16 with fp32 PSUM accumulation; residual stream fp32.
"""
import sys
sys.path.insert(0, "/opt/trn_rl_repo")

import numpy as np
import ml_dtypes

B, HOR, PFX, NO, NR = 8, 10, 16, 128, 8
D, NH, HD, F, L = 768, 12, 64, 3072, 12
TPS = NO + NR
T = PFX + HOR * TPS          # 1376
TP = 1536                    # padded tokens (12 tiles of 128)
NT = TP // 128               # 12 token tiles
DC = D // 128                # 6 d-chunks
EPS = 1e-6
NEG = -30000.0

_CACHE = {}


def _build_nc(n_layers):
    from concourse import bacc
    import concourse.bass as bass
    import concourse.mybir as mybir
    import concourse.tile as tile
    from contextlib import ExitStack

    bf16, f32 = mybir.dt.bfloat16, mybir.dt.float32
    AF = mybir.ActivationFunctionType
    OP = mybir.AluOpType

    nc = bacc.Bacc("TRN2", num_devices=8)

    xT_d = nc.dram_tensor("xT", [D, TP], f32, kind="ExternalInput")
    wqkv_d = nc.dram_tensor("wqkv", [n_layers, D, 3 * D], bf16, kind="ExternalInput")
    bqkvT_d = nc.dram_tensor("bqkvT", [n_layers, 128, 12], f32, kind="ExternalInput")
    bvrow_d = nc.dram_tensor("bvrow", [n_layers, 1, D], bf16, kind="ExternalInput")
    wo_d = nc.dram_tensor("wo", [n_layers, D, D], bf16, kind="ExternalInput")
    borow_d = nc.dram_tensor("borow", [n_layers, 1, D], bf16, kind="ExternalInput")
    w1_d = nc.dram_tensor("w1", [n_layers, D, F], bf16, kind="ExternalInput")
    b1T_d = nc.dram_tensor("b1T", [n_layers, 128, 24], f32, kind="ExternalInput")
    w2_d = nc.dram_tensor("w2", [n_layers, F, D], bf16, kind="ExternalInput")
    b2row_d = nc.dram_tensor("b2row", [n_layers, 1, D], bf16, kind="ExternalInput")
    ln1sT_d = nc.dram_tensor("ln1sT", [n_layers, 128, DC], f32, kind="ExternalInput")
    ln1bT_d = nc.dram_tensor("ln1bT", [n_layers, 128, DC], f32, kind="ExternalInput")
    ln2sT_d = nc.dram_tensor("ln2sT", [n_layers, 128, DC], f32, kind="ExternalInput")
    ln2bT_d = nc.dram_tensor("ln2bT", [n_layers, 128, DC], f32, kind="ExternalInput")
    lnfsT_d = nc.dram_tensor("lnfsT", [128, DC], f32, kind="ExternalInput")
    lnfbT_d = nc.dram_tensor("lnfbT", [128, DC], f32, kind="ExternalInput")
    maskcol_d = nc.dram_tensor("maskcol", [128, NT], f32, kind="ExternalInput")
    rrmask_d = nc.dram_tensor("rrmask", [128, 128], bf16, kind="ExternalInput")
    outT_d = nc.dram_tensor("outT", [D, TP], f32, kind="ExternalOutput")

    def dchunked(ap):  # [D, N] dram AP -> [128, DC-chunks, N]
        return ap.rearrange("(ko p) n -> p ko n", p=128)

    with tile.TileContext(nc) as tc, ExitStack() as ctx:
        const = ctx.enter_context(tc.tile_pool(name="const", bufs=1))
        persist = ctx.enter_context(tc.tile_pool(name="persist", bufs=1))
        wpool = ctx.enter_context(tc.tile_pool(name="wpool", bufs=3))
        bpool = ctx.enter_context(tc.tile_pool(name="bpool", bufs=2))
        rowpool = ctx.enter_context(tc.tile_pool(name="rowpool", bufs=1))
        bigpool = ctx.enter_context(tc.tile_pool(name="bigpool", bufs=1))
        qkpool = ctx.enter_context(tc.tile_pool(name="qkpool", bufs=1))
        lnpool = ctx.enter_context(tc.tile_pool(name="lnpool", bufs=2))
        mupool = ctx.enter_context(tc.tile_pool(name="mupool", bufs=1))
        lnbig = ctx.enter_context(tc.tile_pool(name="lnbig", bufs=1))
        recpool = ctx.enter_context(tc.tile_pool(name="recpool", bufs=1))
        ps_big = ctx.enter_context(tc.tile_pool(name="ps_big", bufs=2, space="PSUM"))
        ps_ot = ctx.enter_context(tc.tile_pool(name="ps_ot", bufs=2, space="PSUM"))

        def ps_mm_tile():
            tmm = ps_big.tile([128, TP], f32, tag="big")
            return tmm

        # ---- persistent state ----
        xT = persist.tile([128, DC, TP], f32)         # residual stream (transposed)
        nc.sync.dma_start(xT, dchunked(xT_d[:]))
        vone = persist.tile([128, NT, NH * 128], bf16)  # per head: [V_h | ones]
        nc.vector.memset(vone, 1.0)
        ARENA_OFF = []
        _o = 0
        for kt in range(NT):
            ARENA_OFF.append(_o)
            _o += TP - (0 if kt == 0 else 128 * kt)
        ptarena = persist.tile([128, _o], bf16)   # exp(scores) arena, [128, 9984]

        # ---- constants ----
        maskcol = const.tile([128, NT], f32)
        nc.sync.dma_start(maskcol, maskcol_d[:])
        rrm = const.tile([128, 128], bf16)
        nc.sync.dma_start(rrm, rrmask_d[:])
        onesPP = const.tile([128, 128], bf16)
        nc.vector.memset(onesPP, 1.0)
        onerow = const.tile([1, 512], bf16)
        nc.vector.memset(onerow, 1.0)
        epsT = const.tile([128, 1], f32)
        nc.vector.memset(epsT, EPS)
        lnfs = const.tile([128, DC], f32)
        nc.sync.dma_start(lnfs, lnfsT_d[:])
        lnfb = const.tile([128, DC], f32)
        nc.sync.dma_start(lnfb, lnfbT_d[:])

        def ln_apply(sT, bT, out_tile):
            """out_tile[:, ko, t] = ln(xT)[:, ko, t] * sT[:, ko] + bT[:, ko]"""
            for c in range(3):
                sl = slice(c * 512, (c + 1) * 512)
                xb = lnbig.tile([128, DC, 512], bf16, tag="xb")
                nc.scalar.activation(xb, xT[:, :, sl], AF.Copy)
                xsq = lnbig.tile([128, DC, 512], bf16, tag="xsq")
                nc.scalar.activation(xsq, xb, AF.Square)
                sums = ps_mm_tile()[:, 0:512]
                for ko in range(DC):
                    nc.tensor.matmul(sums, onesPP, xb[:, ko, :],
                                     start=(ko == 0), stop=(ko == DC - 1))
                sumsq = ps_mm_tile()[:, 0:512]
                for ko in range(DC):
                    nc.tensor.matmul(sumsq, onesPP, xsq[:, ko, :],
                                     start=(ko == 0), stop=(ko == DC - 1))
                mu = mupool.tile([128, 512], f32, tag="mu")
                nc.vector.tensor_scalar_mul(mu, sums, 1.0 / D)
                t = lnpool.tile([128, 512], f32, tag="lntmp")
                nc.vector.tensor_mul(t, mu, sums)
                v = lnpool.tile([128, 512], f32, tag="lntmp")
                nc.vector.tensor_tensor(v, sumsq, t, OP.subtract)
                sd = lnpool.tile([128, 512], f32, tag="lntmp")
                nc.scalar.activation(sd, v, AF.Sqrt, bias=epsT, scale=1.0 / D)
                rstd = lnpool.tile([128, 512], f32, tag="lntmp")
                nc.vector.reciprocal_approx_fast(out=rstd, in_=sd)
                c1 = lnbig.tile([128, DC, 512], f32, tag="c1")
                nc.vector.tensor_tensor(
                    c1, xT[:, :, sl],
                    mu[:, None, :].to_broadcast((128, DC, 512)), OP.subtract)
                nc.vector.tensor_tensor(
                    c1, c1, rstd[:, None, :].to_broadcast((128, DC, 512)), OP.mult)
                for ko in range(DC):
                    nc.vector.tensor_scalar(
                        out=out_tile[:, ko, sl], in0=c1[:, ko, :],
                        scalar1=sT[:, ko:ko + 1], scalar2=bT[:, ko:ko + 1],
                        op0=OP.mult, op1=OP.add)

        def load_w512(dram_ap_chunked, cols):
            """load [128, DC-ish, cols] bf16 weight chunk"""
            n = cols.stop - cols.start
            kdim = dram_ap_chunked.shape[1]
            wt = wpool.tile([128, kdim, n], bf16, tag="w512")
            nc.sync.dma_start(wt, dram_ap_chunked[:, :, cols])
            return wt

        for l in range(n_layers):
            # ---------- LN1 ----------
            s1 = bpool.tile([128, DC], f32, tag="lns")
            nc.sync.dma_start(s1, ln1sT_d[l])
            b1_ = bpool.tile([128, DC], f32, tag="lns")
            nc.sync.dma_start(b1_, ln1bT_d[l])
            yT = bigpool.tile([128, DC, TP], bf16, tag="yT")
            ln_apply(s1, b1_, yT)

            # ---------- QKV ----------
            wq_ch = dchunked(wqkv_d[l])
            bqkv = bpool.tile([128, 12], f32, tag="bqkv")
            nc.sync.dma_start(bqkv, bqkvT_d[l])
            bvrow = rowpool.tile([1, D], bf16, tag="brow")
            nc.sync.dma_start(bvrow, bvrow_d[l])

            # V: natural layout -> vone slots  (out tokens on partitions)
            wv_a = load_w512(wq_ch, slice(1536, 2048))
            wv_b = load_w512(wq_ch, slice(2048, 2304))
            for tt in range(NT):
                for (wt, c0, cl, h0, hn) in ((wv_a, 0, 512, 0, 8), (wv_b, 512, 256, 8, 4)):
                    pv = ps_mm_tile()[:, :cl]
                    for ko in range(DC):
                        nc.tensor.matmul(pv, yT[:, ko, tt * 128:(tt + 1) * 128],
                                         wt[:, ko, :], start=(ko == 0), stop=False)
                    nc.tensor.matmul(pv, onerow[:, 0:128], bvrow[:, c0:c0 + cl],
                                     start=False, stop=True)
                    vslots = vone[:, tt, :].rearrange("p (h s) -> p h s", s=128)
                    nc.vector.tensor_copy(
                        vslots[:, h0:h0 + hn, 0:64],
                        pv.rearrange("p (h s) -> p h s", s=64))

            # QK per head pair + attention
            wq_tiles = [load_w512(wq_ch, slice(512 * i, 512 * (i + 1)))
                        for i in range(3)]
            OT = bigpool.tile([128, DC, TP], bf16, tag="OT")
            for pair in range(6):
                qk = qkpool.tile([128, 2, TP], bf16, tag="qk")
                for i, m in enumerate((pair, 6 + pair)):
                    wt = wq_tiles[(m * 128) // 512]
                    for c in range(3):
                        ps = ps_mm_tile()[:, 0:512]
                        coff = (m * 128) % 512
                        for ko in range(DC):
                            nc.tensor.matmul(ps, wt[:, ko, coff:coff + 128],
                                             yT[:, ko, c * 512:(c + 1) * 512],
                                             start=(ko == 0), stop=(ko == DC - 1))
                        nc.vector.tensor_scalar_add(qk[:, i, c * 512:(c + 1) * 512],
                                                    ps, bqkv[:, m:m + 1])
                for e in range(2):
                    h = 2 * pair + e
                    QT = qk[64 * e:64 * e + 64, 0, :]
                    KT = qk[64 * e:64 * e + 64, 1, :]
                    # ST + exp per key tile into the PT arena
                    for kt in range(NT):
                        qs = 0 if kt == 0 else 128 * kt
                        nq = TP - qs
                        off = ARENA_OFF[kt]
                        st = ps_big.tile([128, nq], f32, tag="big")
                        for g0 in range(qs, TP, 512):
                            g1 = min(g0 + 512, TP)
                            nc.tensor.matmul(st[:, g0 - qs:g1 - qs],
                                             KT[:, kt * 128:(kt + 1) * 128],
                                             QT[:, g0:g1], start=True, stop=True,
                                             skip_group_check=(g0 > qs))
                        pslice = ptarena[:, off:off + nq]
                        nc.scalar.activation(out=pslice, in_=st, func=AF.Exp,
                                             bias=maskcol[:, kt:kt + 1], scale=0.125)
                        if 2 <= kt <= 10:
                            u = kt - 1
                            nc.vector.memset(
                                ptarena[:, off + 1408 - qs:off + 1408 - qs + 8 * u], 0.0)
                        if kt == 11:
                            nc.vector.tensor_mul(pslice, pslice, rrm)
                    for qc in range(3):
                        c0, c1_ = qc * 512, (qc + 1) * 512
                        kts = [kt for kt in range(NT)
                               if (0 if kt == 0 else 128 * kt) < c1_]
                        ot = ps_ot.tile([128, 512], f32, tag="ot")
                        for i, kt in enumerate(kts):
                            qs = 0 if kt == 0 else 128 * kt
                            off = ARENA_OFF[kt]
                            lo = max(qs, c0)
                            nc.tensor.matmul(ot[:, lo - c0:512],
                                             vone[:, kt, 128 * h:128 * h + 128],
                                             ptarena[:, off + lo - qs:off + c1_ - qs],
                                             start=(i == 0), stop=(i == len(kts) - 1),
                                             skip_group_check=(i > 0))
                        rec = recpool.tile([128, 1024], f32, tag="rec")
                        nc.vector.reciprocal(rec[64:128, 0:512], ot[64:128, :])
                        nc.vector.tensor_tensor(OT[64 * e:64 * e + 64, pair, c0:c1_],
                                                ot[0:64, :], rec[64:128, 0:512], OP.mult)

            # ---------- WO + residual ----------
            wo_ch = dchunked(wo_d[l])
            wo_a = load_w512(wo_ch, slice(0, 512))
            wo_b = load_w512(wo_ch, slice(512, 768))
            borow = rowpool.tile([1, D], bf16, tag="brow")
            nc.sync.dma_start(borow, borow_d[l])
            for c in range(3):
                for dc in range(DC):
                    wt, coff = (wo_a, dc * 128) if dc < 4 else (wo_b, (dc - 4) * 128)
                    ps = ps_mm_tile()[:, 0:512]
                    for ko in range(DC):
                        nc.tensor.matmul(ps, wt[:, ko, coff:coff + 128],
                                         OT[:, ko, c * 512:(c + 1) * 512],
                                         start=(ko == 0), stop=False)
                    nc.tensor.matmul(ps, borow[:, dc * 128:dc * 128 + 128], onerow,
                                     start=False, stop=True)
                    nc.vector.tensor_tensor(xT[:, dc, c * 512:(c + 1) * 512],
                                            xT[:, dc, c * 512:(c + 1) * 512], ps, OP.add)

            # ---------- LN2 ----------
            s2 = bpool.tile([128, DC], f32, tag="lns")
            nc.sync.dma_start(s2, ln2sT_d[l])
            b2_ = bpool.tile([128, DC], f32, tag="lns")
            nc.sync.dma_start(b2_, ln2bT_d[l])
            yT2 = bigpool.tile([128, DC, TP], bf16, tag="yT")
            ln_apply(s2, b2_, yT2)

            # ---------- FFN (4 quarters of F) ----------
            w1_ch = dchunked(w1_d[l])
            w2_ch = w2_d[l].rearrange("(fo p) n -> p fo n", p=128)
            b1T = bpool.tile([128, 24], f32, tag="b1T")
            nc.sync.dma_start(b1T, b1T_d[l])
            b2row = rowpool.tile([1, D], bf16, tag="brow")
            nc.sync.dma_start(b2row, b2row_d[l])
            for q4 in range(4):
                f0 = 768 * q4
                w1_a = load_w512(w1_ch, slice(f0, f0 + 512))
                w1_b = load_w512(w1_ch, slice(f0 + 512, f0 + 768))
                hT = bigpool.tile([128, DC, TP], bf16, tag="hT")
                for fm in range(6):
                    wt, coff = (w1_a, fm * 128) if fm < 4 else (w1_b, (fm - 4) * 128)
                    for c in range(3):
                        ps = ps_mm_tile()[:, 0:512]
                        for ko in range(DC):
                            nc.tensor.matmul(ps, wt[:, ko, coff:coff + 128],
                                             yT2[:, ko, c * 512:(c + 1) * 512],
                                             start=(ko == 0), stop=(ko == DC - 1))
                        nc.scalar.activation(out=hT[:, fm, c * 512:(c + 1) * 512], in_=ps,
                                             func=AF.Gelu_apprx_tanh,
                                             bias=b1T[:, 6 * q4 + fm:6 * q4 + fm + 1])
                w2q = w2_ch[:, 6 * q4:6 * q4 + 6, :]
                w2_a = wpool.tile([128, 6, 512], bf16, tag="w512")
                nc.sync.dma_start(w2_a, w2q[:, :, 0:512])
                w2_b = wpool.tile([128, 6, 256], bf16, tag="w512")
                nc.sync.dma_start(w2_b, w2q[:, :, 512:768])
                for c in range(3):
                    for dc in range(DC):
                        wt, coff = (w2_a, dc * 128) if dc < 4 else (w2_b, (dc - 4) * 128)
                        ps = ps_mm_tile()[:, 0:512]
                        for fo in range(6):
                            nc.tensor.matmul(ps, wt[:, fo, coff:coff + 128],
                                             hT[:, fo, c * 512:(c + 1) * 512],
                                             start=(fo == 0),
                                             stop=(fo == 5 and q4 != 0))
                        if q4 == 0:
                            nc.tensor.matmul(ps, b2row[:, dc * 128:dc * 128 + 128],
                                             onerow, start=False, stop=True)
                        nc.vector.tensor_tensor(xT[:, dc, c * 512:(c + 1) * 512],
                                                xT[:, dc, c * 512:(c + 1) * 512],
                                                ps, OP.add)

        # ---------- final LN (in place) + store ----------
        ln_apply(lnfs, lnfb, xT)
        nc.sync.dma_start(dchunked(outT_d[:]), xT)

    nc.compile()
    return nc


# ---------------- host-side glue ----------------

def _prep_weights(inputs, n_layers):
    bf = ml_dtypes.bfloat16
    sl = slice(0, n_layers)

    def dT(a):  # [..., 768] -> [..., 128, 6] (d = ko*128 + p)
        return np.ascontiguousarray(np.swapaxes(a.reshape(*a.shape[:-1], DC, 128), -1, -2))

    wqkv = np.ascontiguousarray(inputs["wqkv"][sl]).astype(bf)
    bqkv = np.asarray(inputs["bqkv"][sl], np.float32)
    bqkvT = np.ascontiguousarray(
        np.swapaxes(bqkv[:, :1536].reshape(n_layers, 12, 128), 1, 2))
    bvrow = bqkv[:, 1536:].reshape(n_layers, 1, D).astype(bf)
    w1 = np.ascontiguousarray(inputs["w1"][sl]).astype(bf)
    b1 = np.asarray(inputs["b1"][sl], np.float32)
    b1T = np.ascontiguousarray(np.swapaxes(b1.reshape(n_layers, 24, 128), 1, 2))
    return {
        "wqkv": wqkv,
        "bqkvT": np.ascontiguousarray(bqkvT, np.float32),
        "bvrow": np.ascontiguousarray(bvrow),
        "wo": np.ascontiguousarray(inputs["wo"][sl]).astype(bf),
        "borow": np.asarray(inputs["bo"][sl], np.float32).reshape(n_layers, 1, D).astype(bf),
        "w1": w1,
        "b1T": np.ascontiguousarray(b1T, np.float32),
        "w2": np.ascontiguousarray(inputs["w2"][sl]).astype(bf),
        "b2row": np.asarray(inputs["b2"][sl], np.float32).reshape(n_layers, 1, D).astype(bf),
        "ln1sT": np.ascontiguousarray(dT(np.asarray(inputs["ln1_s"][sl], np.float32))),
        "ln1bT": np.ascontiguousarray(dT(np.asarray(inputs["ln1_b"][sl], np.float32))),
        "ln2sT": np.ascontiguousarray(dT(np.asarray(inputs["ln2_s"][sl], np.float32))),
        "ln2bT": np.ascontiguousarray(dT(np.asarray(inputs["ln2_b"][sl], np.float32))),
        "lnfsT": np.ascontiguousarray(dT(np.asarray(inputs["lnf_s"], np.float32))),
        "lnfbT": np.ascontiguousarray(dT(np.asarray(inputs["lnf_b"], np.float32))),
    }


def _rrmask():
    m = np.zeros((128, 128), np.float32)
    for k in range(80):
        m[k, (k // 8) * 8:] = 1.0
    return m.astype(ml_dtypes.bfloat16)


def _maskcol(prefix_mask, obs_mask, readout_mask):
    """[128, 12] additive exp-bias per (key partition, key tile)."""
    m = np.full((128, NT), NEG, np.float32)
    m[:PFX, 0] = np.where(prefix_mask, 0.0, NEG)
    for t in range(HOR):
        m[:, 1 + t] = np.where(obs_mask[t], 0.0, NEG)
    ro = np.asarray(readout_mask).reshape(-1)
    m[:80, 11] = np.where(ro, 0.0, NEG)
    return m


def _assemble_xT(prefix, obs, readout):
    """(16,768),(10,128,768),(10,8,768) -> transposed padded (768,1536) f32"""
    x = np.zeros((TP, D), np.float32)
    x[:PFX] = prefix
    x[128:128 + HOR * 128] = obs.reshape(HOR * 128, D)
    x[1408:1408 + HOR * NR] = readout.reshape(HOR * NR, D)
    return np.ascontiguousarray(x.T)


def _gather_out(outT):
    """(768,1536) -> (1376, 768) in original token order"""
    xo = outT.T
    out = np.empty((T, D), np.float32)
    out[:PFX] = xo[:PFX]
    for t in range(HOR):
        out[PFX + TPS * t:PFX + TPS * t + NO] = xo[128 * (1 + t):128 * (2 + t)]
        out[PFX + TPS * t + NO:PFX + TPS * (t + 1)] = xo[1408 + NR * t:1408 + NR * (t + 1)]
    return out


def run(inputs, n_layers=L, trace=False, tmpdir=None):
    from concourse.bass_utils import run_bass_kernel_spmd

    key = ("nc", n_layers)
    if key not in _CACHE:
        _CACHE[key] = _build_nc(n_layers)
    nc = _CACHE[key]

    wmap = _prep_weights(inputs, n_layers)
    rr = _rrmask()
    pm = np.asarray(inputs["prefix_mask"], bool)
    om = np.asarray(inputs["obs_mask"], bool)
    rm = np.asarray(inputs["readout_mask"], bool)
    pt = np.asarray(inputs["prefix_tokens"], np.float32)
    ot = np.asarray(inputs["obs_tokens"], np.float32)
    rt = np.asarray(inputs["readout_tokens"], np.float32)

    in_maps = []
    for b in range(B):
        m = dict(wmap)
        m["xT"] = _assemble_xT(pt[b], ot[b], rt[b])
        m["maskcol"] = _maskcol(pm[b], om[b], rm[b])
        m["rrmask"] = rr
        in_maps.append(m)

    res = run_bass_kernel_spmd(nc, in_maps, list(range(B)), trace=trace, tmpdir=tmpdir)
    out = np.stack([_gather_out(res.results[b]["outT"]) for b in range(B)])
    return out, res


def kernel(**inputs):
    out, _ = run(inputs, L)
    return out



# revision 9
# speedup vs baseline: 1.2371x; 1.2371x over previous
"""Trainium2 Bass kernel for nn_BlockTransformer (Octo-style block-sparse transformer).

Strategy: data-parallel over batch (B=8 -> 1 element per NeuronCore), weights
replicated. Residual stream kept transposed (D on partitions) to avoid all
on-device transposes. Tokens reordered to [prefix|pad, obs t0..t9 (128-aligned),
readouts|pad] = 1536 padded tokens so the block-sparse attention mask becomes
128-aligned; per-key mask folded into the softmax-exp bias (per-partition),
readout causality handled by small memsets + one static 0/1 multiplier tile.
Matmuls in bf16 with fp32 PSUM accumulation; residual stream fp32.

v2: LN affine folded into wqkv/w1 host-side; LN normalize in bf16 (2x DVE);
softmax reciprocal via reciprocal_approx_fast; biases for wo/w2 fused into the
residual add (scalar_tensor_tensor); 512-wide PSUM tiles with deep rotation.
"""
import sys
sys.path.insert(0, "/opt/trn_rl_repo")

import numpy as np
import ml_dtypes

B, HOR, PFX, NO, NR = 8, 10, 16, 128, 8
D, NH, HD, F, L = 768, 12, 64, 3072, 12
TPS = NO + NR
T = PFX + HOR * TPS          # 1376
TP = 1536                    # padded tokens (12 tiles of 128)
NT = TP // 128               # 12 token tiles
DC = D // 128                # 6 d-chunks
EPS = 1e-6
NEG = -30000.0

_CACHE = {}


def _build_nc(n_layers):
    from concourse import bacc
    import concourse.bass as bass
    import concourse.mybir as mybir
    import concourse.tile as tile
    from contextlib import ExitStack

    bf16, f32 = mybir.dt.bfloat16, mybir.dt.float32
    AF = mybir.ActivationFunctionType
    OP = mybir.AluOpType

    nc = bacc.Bacc("TRN2", num_devices=8)

    xT_d = nc.dram_tensor("xT", [D, TP], f32, kind="ExternalInput")
    wqkv_d = nc.dram_tensor("wqkv", [n_layers, D, 3 * D], bf16, kind="ExternalInput")
    bqkvT_d = nc.dram_tensor("bqkvT", [n_layers, 128, 12], f32, kind="ExternalInput")
    bvrow_d = nc.dram_tensor("bvrow", [n_layers, 1, D], bf16, kind="ExternalInput")
    wo_d = nc.dram_tensor("wo", [n_layers, D, D], bf16, kind="ExternalInput")
    bocolT_d = nc.dram_tensor("bocolT", [n_layers, 128, DC], f32, kind="ExternalInput")
    w1_d = nc.dram_tensor("w1", [n_layers, D, F], bf16, kind="ExternalInput")
    b1T_d = nc.dram_tensor("b1T", [n_layers, 128, 24], f32, kind="ExternalInput")
    w2_d = nc.dram_tensor("w2", [n_layers, F, D], bf16, kind="ExternalInput")
    b2colT_d = nc.dram_tensor("b2colT", [n_layers, 128, DC], f32, kind="ExternalInput")
    lnfsT_d = nc.dram_tensor("lnfsT", [128, DC], f32, kind="ExternalInput")
    lnfbT_d = nc.dram_tensor("lnfbT", [128, DC], f32, kind="ExternalInput")
    maskcol_d = nc.dram_tensor("maskcol", [128, NT], f32, kind="ExternalInput")
    rrmask_d = nc.dram_tensor("rrmask", [128, 128], bf16, kind="ExternalInput")
    outT_d = nc.dram_tensor("outT", [D, TP], f32, kind="ExternalOutput")

    def dchunked(ap):  # [D, N] dram AP -> [128, DC-chunks, N]
        return ap.rearrange("(ko p) n -> p ko n", p=128)

    with tile.TileContext(nc) as tc, ExitStack() as ctx:
        const = ctx.enter_context(tc.tile_pool(name="const", bufs=1))
        persist = ctx.enter_context(tc.tile_pool(name="persist", bufs=1))
        wpool = ctx.enter_context(tc.tile_pool(name="wpool", bufs=3))
        bpool = ctx.enter_context(tc.tile_pool(name="bpool", bufs=2))
        rowpool = ctx.enter_context(tc.tile_pool(name="rowpool", bufs=1))
        bigpool = ctx.enter_context(tc.tile_pool(name="bigpool", bufs=1))
        qkpool = ctx.enter_context(tc.tile_pool(name="qkpool", bufs=1))
        lnpool = ctx.enter_context(tc.tile_pool(name="lnpool", bufs=2))
        mupool = ctx.enter_context(tc.tile_pool(name="mupool", bufs=2))
        lnbig = ctx.enter_context(tc.tile_pool(name="lnbig", bufs=1))
        recpool = ctx.enter_context(tc.tile_pool(name="recpool", bufs=1))
        ps_g = ctx.enter_context(tc.tile_pool(name="ps_g", bufs=6, space="PSUM"))
        ps_ot = ctx.enter_context(tc.tile_pool(name="ps_ot", bufs=2, space="PSUM"))

        def ps_tile():
            tmm = ps_g.tile([128, 512], f32, tag="g")
            return tmm

        # ---- persistent state ----
        xT = persist.tile([128, DC, TP], f32)         # residual stream (transposed)
        nc.sync.dma_start(xT, dchunked(xT_d[:]))
        vone = persist.tile([128, NT, NH * 128], bf16)  # per head: [V_h | ones]
        nc.vector.memset(vone, 1.0)
        ARENA_OFF = []
        _o = 0
        for kt in range(NT):
            ARENA_OFF.append(_o)
            _o += TP - (0 if kt == 0 else 128 * kt)
        ptarena = persist.tile([128, _o], bf16)   # exp(scores) arena, [128, 9984]

        # ---- constants ----
        maskcol = const.tile([128, NT], f32)
        nc.sync.dma_start(maskcol, maskcol_d[:])
        rrm = const.tile([128, 128], bf16)
        nc.sync.dma_start(rrm, rrmask_d[:])
        onesPP = const.tile([128, 128], bf16)
        nc.vector.memset(onesPP, 1.0)
        onerow = const.tile([1, 512], bf16)
        nc.vector.memset(onerow, 1.0)
        epsT = const.tile([128, 1], f32)
        nc.vector.memset(epsT, EPS)
        lnfs = const.tile([128, DC], f32)
        nc.sync.dma_start(lnfs, lnfsT_d[:])
        lnfb = const.tile([128, DC], f32)
        nc.sync.dma_start(lnfb, lnfbT_d[:])

        def ln_stats(sl):
            """returns (mu_b, rstd_b, xb) for token slice sl (512 wide)."""
            xb = lnbig.tile([128, DC, 512], bf16, tag="xb")
            nc.scalar.activation(xb, xT[:, :, sl], AF.Copy)
            xsq = lnbig.tile([128, DC, 512], bf16, tag="xsq")
            nc.scalar.activation(xsq, xb, AF.Square)
            sums = ps_tile()
            for ko in range(DC):
                nc.tensor.matmul(sums, onesPP, xb[:, ko, :],
                                 start=(ko == 0), stop=(ko == DC - 1))
            sumsq = ps_tile()
            for ko in range(DC):
                nc.tensor.matmul(sumsq, onesPP, xsq[:, ko, :],
                                 start=(ko == 0), stop=(ko == DC - 1))
            mu = mupool.tile([128, 512], f32, tag="mu")
            nc.vector.tensor_scalar_mul(mu, sums, 1.0 / D)
            t = lnpool.tile([128, 512], f32, tag="lntmp")
            nc.vector.tensor_mul(t, mu, sums)
            v = lnpool.tile([128, 512], f32, tag="lntmp")
            nc.vector.tensor_tensor(v, sumsq, t, OP.subtract)
            sd = lnpool.tile([128, 512], f32, tag="lntmp")
            nc.scalar.activation(sd, v, AF.Sqrt, bias=epsT, scale=1.0 / D)
            rstd = lnpool.tile([128, 512], f32, tag="lntmp")
            nc.vector.reciprocal_approx_fast(out=rstd, in_=sd)
            mu_b = mupool.tile([128, 512], bf16, tag="mub")
            nc.vector.tensor_copy(mu_b, mu)
            rstd_b = mupool.tile([128, 512], bf16, tag="rstdb")
            nc.vector.tensor_copy(rstd_b, rstd)
            return mu_b, rstd_b, xb

        def ln_apply(out_tile):
            """out_tile[:, ko, t] = (x - mu) * rstd  (affine folded into weights)"""
            for c in range(3):
                sl = slice(c * 512, (c + 1) * 512)
                mu_b, rstd_b, xb = ln_stats(sl)
                nc.vector.tensor_tensor(
                    out_tile[:, :, sl], xb,
                    mu_b[:, None, :].to_broadcast((128, DC, 512)), OP.subtract)
                nc.vector.tensor_tensor(
                    out_tile[:, :, sl], out_tile[:, :, sl],
                    rstd_b[:, None, :].to_broadcast((128, DC, 512)), OP.mult)

        def ln_final(sT, bT):
            """final LN with affine, normalized part in bf16, in-place on xT."""
            c1 = bigpool.tile([128, DC, TP], bf16, tag="yT")
            for c in range(3):
                sl = slice(c * 512, (c + 1) * 512)
                mu_b, rstd_b, xb = ln_stats(sl)
                nc.vector.tensor_tensor(
                    c1[:, :, sl], xb,
                    mu_b[:, None, :].to_broadcast((128, DC, 512)), OP.subtract)
                nc.vector.tensor_tensor(
                    c1[:, :, sl], c1[:, :, sl],
                    rstd_b[:, None, :].to_broadcast((128, DC, 512)), OP.mult)
                for ko in range(DC):
                    nc.vector.tensor_scalar(
                        out=xT[:, ko, sl], in0=c1[:, ko, sl],
                        scalar1=sT[:, ko:ko + 1], scalar2=bT[:, ko:ko + 1],
                        op0=OP.mult, op1=OP.add)

        def load_w512(dram_ap_chunked, cols):
            """load [128, DC-ish, cols] bf16 weight chunk"""
            n = cols.stop - cols.start
            kdim = dram_ap_chunked.shape[1]
            wt = wpool.tile([128, kdim, n], bf16, tag="w512")
            nc.sync.dma_start(wt, dram_ap_chunked[:, :, cols])
            return wt

        for l in range(n_layers):
            # ---------- LN1 (affine folded into wqkv/bqkv) ----------
            yT = bigpool.tile([128, DC, TP], bf16, tag="yT")
            ln_apply(yT)

            # ---------- QKV ----------
            wq_ch = dchunked(wqkv_d[l])
            bqkv = bpool.tile([128, 12], f32, tag="bqkv")
            nc.sync.dma_start(bqkv, bqkvT_d[l])
            bvrow = rowpool.tile([1, D], bf16, tag="brow")
            nc.sync.dma_start(bvrow, bvrow_d[l])

            # V: natural layout -> vone slots  (out tokens on partitions)
            wv_a = load_w512(wq_ch, slice(1536, 2048))
            wv_b = load_w512(wq_ch, slice(2048, 2304))
            for tt in range(NT):
                for (wt, c0, cl, h0, hn) in ((wv_a, 0, 512, 0, 8), (wv_b, 512, 256, 8, 4)):
                    pv = ps_tile()[:, :cl]
                    for ko in range(DC):
                        nc.tensor.matmul(pv, yT[:, ko, tt * 128:(tt + 1) * 128],
                                         wt[:, ko, :], start=(ko == 0), stop=False)
                    nc.tensor.matmul(pv, onerow[:, 0:128], bvrow[:, c0:c0 + cl],
                                     start=False, stop=True)
                    vslots = vone[:, tt, :].rearrange("p (h s) -> p h s", s=128)
                    nc.vector.tensor_copy(
                        vslots[:, h0:h0 + hn, 0:64],
                        pv.rearrange("p (h s) -> p h s", s=64))

            # QK per head pair + attention
            wq_tiles = [load_w512(wq_ch, slice(512 * i, 512 * (i + 1)))
                        for i in range(3)]
            OT = bigpool.tile([128, DC, TP], bf16, tag="OT")
            for pair in range(6):
                qk = qkpool.tile([128, 2, TP], bf16, tag="qk")
                for i, m in enumerate((pair, 6 + pair)):
                    wt = wq_tiles[(m * 128) // 512]
                    for c in range(3):
                        ps = ps_tile()
                        coff = (m * 128) % 512
                        for ko in range(DC):
                            nc.tensor.matmul(ps, wt[:, ko, coff:coff + 128],
                                             yT[:, ko, c * 512:(c + 1) * 512],
                                             start=(ko == 0), stop=(ko == DC - 1))
                        nc.vector.tensor_scalar_add(qk[:, i, c * 512:(c + 1) * 512],
                                                    ps, bqkv[:, m:m + 1])
                for e in range(2):
                    h = 2 * pair + e
                    QT = qk[64 * e:64 * e + 64, 0, :]
                    KT = qk[64 * e:64 * e + 64, 1, :]
                    # ST + exp per key tile into the PT arena (512-wide psum segs)
                    for kt in range(NT):
                        qs = 0 if kt == 0 else 128 * kt
                        off = ARENA_OFF[kt]
                        for g0 in range(qs, TP, 512):
                            g1 = min(g0 + 512, TP)
                            st = ps_tile()[:, 0:g1 - g0]
                            nc.tensor.matmul(st,
                                             KT[:, kt * 128:(kt + 1) * 128],
                                             QT[:, g0:g1], start=True, stop=True)
                            pslice = ptarena[:, off + g0 - qs:off + g1 - qs]
                            nc.scalar.activation(out=pslice, in_=st, func=AF.Exp,
                                                 bias=maskcol[:, kt:kt + 1], scale=0.125)
                        if 2 <= kt <= 10:
                            u = kt - 1
                            nc.vector.memset(
                                ptarena[:, off + 1408 - qs:off + 1408 - qs + 8 * u], 0.0)
                        if kt == 11:
                            nc.vector.tensor_mul(ptarena[:, off:off + 128],
                                                 ptarena[:, off:off + 128], rrm)
                    for qc in range(3):
                        c0, c1_ = qc * 512, (qc + 1) * 512
                        kts = [kt for kt in range(NT)
                               if (0 if kt == 0 else 128 * kt) < c1_]
                        ot = ps_ot.tile([128, 512], f32, tag="ot")
                        for i, kt in enumerate(kts):
                            qs = 0 if kt == 0 else 128 * kt
                            off = ARENA_OFF[kt]
                            lo = max(qs, c0)
                            nc.tensor.matmul(ot[:, lo - c0:512],
                                             vone[:, kt, 128 * h:128 * h + 128],
                                             ptarena[:, off + lo - qs:off + c1_ - qs],
                                             start=(i == 0), stop=(i == len(kts) - 1),
                                             skip_group_check=(i > 0))
                        rec = recpool.tile([128, 1024], f32, tag="rec")
                        nc.vector.tensor_copy(rec[0:64, 512:1024], ot[64:128, :])
                        nc.vector.reciprocal_approx_fast(
                            out=rec[0:64, 0:512], in_=rec[0:64, 512:1024])
                        nc.vector.tensor_tensor(OT[64 * e:64 * e + 64, pair, c0:c1_],
                                                ot[0:64, :], rec[0:64, 0:512], OP.mult)

            # ---------- WO + residual (bias fused into residual add) ----------
            wo_ch = dchunked(wo_d[l])
            wo_a = load_w512(wo_ch, slice(0, 512))
            wo_b = load_w512(wo_ch, slice(512, 768))
            bocol = bpool.tile([128, DC], f32, tag="bocol")
            nc.sync.dma_start(bocol, bocolT_d[l])
            for c in range(3):
                for dc in range(DC):
                    wt, coff = (wo_a, dc * 128) if dc < 4 else (wo_b, (dc - 4) * 128)
                    ps = ps_tile()
                    for ko in range(DC):
                        nc.tensor.matmul(ps, wt[:, ko, coff:coff + 128],
                                         OT[:, ko, c * 512:(c + 1) * 512],
                                         start=(ko == 0), stop=(ko == DC - 1))
                    nc.vector.scalar_tensor_tensor(
                        out=xT[:, dc, c * 512:(c + 1) * 512],
                        in0=ps, scalar=bocol[:, dc:dc + 1],
                        in1=xT[:, dc, c * 512:(c + 1) * 512],
                        op0=OP.add, op1=OP.add)

            # ---------- LN2 (affine folded into w1/b1) ----------
            yT2 = bigpool.tile([128, DC, TP], bf16, tag="yT")
            ln_apply(yT2)

            # ---------- FFN (4 quarters of F) ----------
            w1_ch = dchunked(w1_d[l])
            w2_ch = w2_d[l].rearrange("(fo p) n -> p fo n", p=128)
            b1T = bpool.tile([128, 24], f32, tag="b1T")
            nc.sync.dma_start(b1T, b1T_d[l])
            b2col = bpool.tile([128, DC], f32, tag="b2col")
            nc.sync.dma_start(b2col, b2colT_d[l])
            for q4 in range(4):
                f0 = 768 * q4
                w1_a = load_w512(w1_ch, slice(f0, f0 + 512))
                w1_b = load_w512(w1_ch, slice(f0 + 512, f0 + 768))
                hT = bigpool.tile([128, DC, TP], bf16, tag="hT")
                for fm in range(6):
                    wt, coff = (w1_a, fm * 128) if fm < 4 else (w1_b, (fm - 4) * 128)
                    for c in range(3):
                        ps = ps_tile()
                        for ko in range(DC):
                            nc.tensor.matmul(ps, wt[:, ko, coff:coff + 128],
                                             yT2[:, ko, c * 512:(c + 1) * 512],
                                             start=(ko == 0), stop=(ko == DC - 1))
                        nc.scalar.activation(out=hT[:, fm, c * 512:(c + 1) * 512], in_=ps,
                                             func=AF.Gelu_apprx_tanh,
                                             bias=b1T[:, 6 * q4 + fm:6 * q4 + fm + 1])
                w2q = w2_ch[:, 6 * q4:6 * q4 + 6, :]
                w2_a = wpool.tile([128, 6, 512], bf16, tag="w512")
                nc.sync.dma_start(w2_a, w2q[:, :, 0:512])
                w2_b = wpool.tile([128, 6, 256], bf16, tag="w512")
                nc.sync.dma_start(w2_b, w2q[:, :, 512:768])
                for c in range(3):
                    for dc in range(DC):
                        wt, coff = (w2_a, dc * 128) if dc < 4 else (w2_b, (dc - 4) * 128)
                        ps = ps_tile()
                        for fo in range(6):
                            nc.tensor.matmul(ps, wt[:, fo, coff:coff + 128],
                                             hT[:, fo, c * 512:(c + 1) * 512],
                                             start=(fo == 0), stop=(fo == 5))
                        if q4 == 0:
                            nc.vector.scalar_tensor_tensor(
                                out=xT[:, dc, c * 512:(c + 1) * 512],
                                in0=ps, scalar=b2col[:, dc:dc + 1],
                                in1=xT[:, dc, c * 512:(c + 1) * 512],
                                op0=OP.add, op1=OP.add)
                        else:
                            nc.vector.tensor_tensor(
                                xT[:, dc, c * 512:(c + 1) * 512],
                                xT[:, dc, c * 512:(c + 1) * 512],
                                ps, OP.add)

        # ---------- final LN (in place) + store ----------
        ln_final(lnfs, lnfb)
        nc.sync.dma_start(dchunked(outT_d[:]), xT)

    nc.compile()
    return nc


# ---------------- host-side glue ----------------

def _prep_weights(inputs, n_layers):
    bf = ml_dtypes.bfloat16
    sl = slice(0, n_layers)

    def dT(a):  # [..., 768] -> [..., 128, 6] (d = ko*128 + p)
        return np.ascontiguousarray(np.swapaxes(a.reshape(*a.shape[:-1], DC, 128), -1, -2))

    ln1s = np.asarray(inputs["ln1_s"][sl], np.float32)
    ln1b = np.asarray(inputs["ln1_b"][sl], np.float32)
    ln2s = np.asarray(inputs["ln2_s"][sl], np.float32)
    ln2b = np.asarray(inputs["ln2_b"][sl], np.float32)

    # fold LN1 affine into wqkv/bqkv, LN2 affine into w1/b1
    wqkv = np.asarray(inputs["wqkv"][sl], np.float32)
    bqkv = np.asarray(inputs["bqkv"][sl], np.float32)
    wqkv_f = ln1s[:, :, None] * wqkv
    bqkv_f = bqkv + np.einsum('ld,lde->le', ln1b, wqkv)
    w1 = np.asarray(inputs["w1"][sl], np.float32)
    b1 = np.asarray(inputs["b1"][sl], np.float32)
    w1_f = ln2s[:, :, None] * w1
    b1_f = b1 + np.einsum('ld,lde->le', ln2b, w1)

    bqkvT = np.ascontiguousarray(
        np.swapaxes(bqkv_f[:, :1536].reshape(n_layers, 12, 128), 1, 2))
    bvrow = bqkv_f[:, 1536:].reshape(n_layers, 1, D).astype(bf)
    b1T = np.ascontiguousarray(np.swapaxes(b1_f.reshape(n_layers, 24, 128), 1, 2))
    return {
        "wqkv": np.ascontiguousarray(wqkv_f).astype(bf),
        "bqkvT": np.ascontiguousarray(bqkvT, np.float32),
        "bvrow": np.ascontiguousarray(bvrow),
        "wo": np.ascontiguousarray(inputs["wo"][sl]).astype(bf),
        "bocolT": np.ascontiguousarray(dT(np.asarray(inputs["bo"][sl], np.float32))),
        "w1": np.ascontiguousarray(w1_f).astype(bf),
        "b1T": np.ascontiguousarray(b1T, np.float32),
        "w2": np.ascontiguousarray(inputs["w2"][sl]).astype(bf),
        "b2colT": np.ascontiguousarray(dT(np.asarray(inputs["b2"][sl], np.float32))),
        "lnfsT": np.ascontiguousarray(dT(np.asarray(inputs["lnf_s"], np.float32))),
        "lnfbT": np.ascontiguousarray(dT(np.asarray(inputs["lnf_b"], np.float32))),
    }


def _rrmask():
    m = np.zeros((128, 128), np.float32)
    for k in range(80):
        m[k, (k // 8) * 8:] = 1.0
    return m.astype(ml_dtypes.bfloat16)


def _maskcol(prefix_mask, obs_mask, readout_mask):
    """[128, 12] additive exp-bias per (key partition, key tile)."""
    m = np.full((128, NT), NEG, np.float32)
    m[:PFX, 0] = np.where(prefix_mask, 0.0, NEG)
    for t in range(HOR):
        m[:, 1 + t] = np.where(obs_mask[t], 0.0, NEG)
    ro = np.asarray(readout_mask).reshape(-1)
    m[:80, 11] = np.where(ro, 0.0, NEG)
    return m


def _assemble_xT(prefix, obs, readout):
    """(16,768),(10,128,768),(10,8,768) -> transposed padded (768,1536) f32"""
    x = np.zeros((TP, D), np.float32)
    x[:PFX] = prefix
    x[128:128 + HOR * 128] = obs.reshape(HOR * 128, D)
    x[1408:1408 + HOR * NR] = readout.reshape(HOR * NR, D)
    return np.ascontiguousarray(x.T)


def _gather_out(outT):
    """(768,1536) -> (1376, 768) in original token order"""
    xo = outT.T
    out = np.empty((T, D), np.float32)
    out[:PFX] = xo[:PFX]
    for t in range(HOR):
        out[PFX + TPS * t:PFX + TPS * t + NO] = xo[128 * (1 + t):128 * (2 + t)]
        out[PFX + TPS * t + NO:PFX + TPS * (t + 1)] = xo[1408 + NR * t:1408 + NR * (t + 1)]
    return out


def run(inputs, n_layers=L, trace=False, tmpdir=None):
    from concourse.bass_utils import run_bass_kernel_spmd

    key = ("nc", n_layers)
    if key not in _CACHE:
        _CACHE[key] = _build_nc(n_layers)
    nc = _CACHE[key]

    wmap = _prep_weights(inputs, n_layers)
    rr = _rrmask()
    pm = np.asarray(inputs["prefix_mask"], bool)
    om = np.asarray(inputs["obs_mask"], bool)
    rm = np.asarray(inputs["readout_mask"], bool)
    pt = np.asarray(inputs["prefix_tokens"], np.float32)
    ot = np.asarray(inputs["obs_tokens"], np.float32)
    rt = np.asarray(inputs["readout_tokens"], np.float32)

    in_maps = []
    for b in range(B):
        m = dict(wmap)
        m["xT"] = _assemble_xT(pt[b], ot[b], rt[b])
        m["maskcol"] = _maskcol(pm[b], om[b], rm[b])
        m["rrmask"] = rr
        in_maps.append(m)

    res = run_bass_kernel_spmd(nc, in_maps, list(range(B)), trace=trace, tmpdir=tmpdir)
    out = np.stack([_gather_out(res.results[b]["outT"]) for b in range(B)])
    return out, res


def kernel(**inputs):
    out, _ = run(inputs, L)
    return out


# revision 16
# speedup vs baseline: 1.2900x; 1.0428x over previous
"""Trainium2 Bass kernel for nn_BlockTransformer (Octo-style block-sparse transformer).

Strategy: data-parallel over batch (B=8 -> 1 element per NeuronCore), weights
replicated. Residual stream kept transposed (D on partitions) to avoid all
on-device transposes. Tokens reordered to [prefix|pad, obs t0..t9 (128-aligned),
readouts|pad] = 1536 padded tokens so the block-sparse attention mask becomes
128-aligned; per-key mask folded into the softmax-exp bias (per-partition),
readout causality handled by small memsets + one static 0/1 multiplier tile.
Matmuls in bf16 with fp32 PSUM accumulation; residual stream fp32.

v2: LN affine folded into wqkv/w1 host-side; LN normalize in bf16 (2x DVE);
softmax reciprocal via reciprocal_approx_fast; biases for wo/w2 fused into the
residual add (scalar_tensor_tensor); 512-wide PSUM tiles with deep rotation.
"""
import sys
sys.path.insert(0, "/opt/trn_rl_repo")

import numpy as np
import ml_dtypes

B, HOR, PFX, NO, NR = 8, 10, 16, 128, 8
D, NH, HD, F, L = 768, 12, 64, 3072, 12
TPS = NO + NR
T = PFX + HOR * TPS          # 1376
TP = 1536                    # padded tokens (12 tiles of 128)
NT = TP // 128               # 12 token tiles
DC = D // 128                # 6 d-chunks
EPS = 1e-6
NEG = -30000.0

_CACHE = {}


def _build_nc(n_layers):
    from concourse import bacc
    import concourse.bass as bass
    import concourse.mybir as mybir
    import concourse.tile as tile
    from contextlib import ExitStack

    bf16, f32, f8 = mybir.dt.bfloat16, mybir.dt.float32, mybir.dt.float8e4
    AF = mybir.ActivationFunctionType
    OP = mybir.AluOpType

    nc = bacc.Bacc("TRN2", num_devices=8)

    xT_d = nc.dram_tensor("xT", [D, TP], f32, kind="ExternalInput")
    wqkv_d = nc.dram_tensor("wqkv", [n_layers, D, 3 * D], bf16, kind="ExternalInput")
    bqkvT_d = nc.dram_tensor("bqkvT", [n_layers, 128, 12], f32, kind="ExternalInput")
    wo_d = nc.dram_tensor("wo", [n_layers, D, D], bf16, kind="ExternalInput")
    bocolT_d = nc.dram_tensor("bocolT", [n_layers, 128, DC], f32, kind="ExternalInput")
    w1_d = nc.dram_tensor("w1", [n_layers, D, F], bf16, kind="ExternalInput")
    b1T_d = nc.dram_tensor("b1T", [n_layers, 128, 24], f32, kind="ExternalInput")
    w2_d = nc.dram_tensor("w2", [n_layers, F, D], bf16, kind="ExternalInput")
    b2colT_d = nc.dram_tensor("b2colT", [n_layers, 128, DC], f32, kind="ExternalInput")
    lnfsT_d = nc.dram_tensor("lnfsT", [128, DC], f32, kind="ExternalInput")
    lnfbT_d = nc.dram_tensor("lnfbT", [128, DC], f32, kind="ExternalInput")
    maskcol_d = nc.dram_tensor("maskcol", [128, NT], f32, kind="ExternalInput")
    rrmask_d = nc.dram_tensor("rrmask", [128, 128], bf16, kind="ExternalInput")
    outT_d = nc.dram_tensor("outT", [D, TP], f32, kind="ExternalOutput")

    def dchunked(ap):  # [D, N] dram AP -> [128, DC-chunks, N]
        return ap.rearrange("(ko p) n -> p ko n", p=128)

    with tile.TileContext(nc) as tc, ExitStack() as ctx:
        const = ctx.enter_context(tc.tile_pool(name="const", bufs=1))
        persist = ctx.enter_context(tc.tile_pool(name="persist", bufs=1))
        wpool = ctx.enter_context(tc.tile_pool(name="wpool", bufs=3))
        bpool = ctx.enter_context(tc.tile_pool(name="bpool", bufs=2))
        rowpool = ctx.enter_context(tc.tile_pool(name="rowpool", bufs=1))
        bigpool = ctx.enter_context(tc.tile_pool(name="bigpool", bufs=1))
        qkpool = ctx.enter_context(tc.tile_pool(name="qkpool", bufs=2))
        lnpool = ctx.enter_context(tc.tile_pool(name="lnpool", bufs=2))
        mupool = ctx.enter_context(tc.tile_pool(name="mupool", bufs=2))
        lnbig = ctx.enter_context(tc.tile_pool(name="lnbig", bufs=1))
        recpool = ctx.enter_context(tc.tile_pool(name="recpool", bufs=1))
        ps_g = ctx.enter_context(tc.tile_pool(name="ps_g", bufs=6, space="PSUM"))
        ps_ot = ctx.enter_context(tc.tile_pool(name="ps_ot", bufs=2, space="PSUM"))

        def ps_tile():
            tmm = ps_g.tile([128, 512], f32, tag="g")
            return tmm

        # ---- persistent state ----
        xT = persist.tile([128, DC, TP], f32)         # residual stream (transposed)
        nc.sync.dma_start(xT, dchunked(xT_d[:]))
        vone = persist.tile([128, NT, NH * 128], bf16)  # per head: [V_h | ones]
        nc.vector.memset(vone, 1.0)
        ARENA_OFF = []
        _o = 0
        for kt in range(NT):
            ARENA_OFF.append(_o)
            _o += TP - (0 if kt == 0 else 128 * kt)
        # two fp8 exp(score) arenas -> both heads of a pair live concurrently
        ptarena2 = persist.tile([128, 2, _o], f8, tag="ptarena2")

        # ---- constants ----
        maskcol = const.tile([128, NT], f32)
        nc.sync.dma_start(maskcol, maskcol_d[:])
        rrm = const.tile([128, 128], bf16)
        nc.sync.dma_start(rrm, rrmask_d[:])
        onesPP = const.tile([128, 128], bf16)
        nc.vector.memset(onesPP, 1.0)
        onerow = const.tile([1, 512], bf16)
        nc.vector.memset(onerow, 1.0)
        epsT = const.tile([128, 1], f32)
        nc.vector.memset(epsT, EPS)
        lnfs = const.tile([128, DC], f32)
        nc.sync.dma_start(lnfs, lnfsT_d[:])
        lnfb = const.tile([128, DC], f32)
        nc.sync.dma_start(lnfb, lnfbT_d[:])

        def ln_stats(sl):
            """returns (mu_b, rstd_b, xb) for token slice sl (512 wide)."""
            xb = lnbig.tile([128, DC, 512], bf16, tag="xb")
            nc.scalar.activation(xb, xT[:, :, sl], AF.Copy)
            xsq = lnbig.tile([128, DC, 512], bf16, tag="xsq")
            nc.scalar.activation(xsq, xb, AF.Square)
            sums = ps_tile()
            for ko in range(DC):
                nc.tensor.matmul(sums, onesPP, xb[:, ko, :],
                                 start=(ko == 0), stop=(ko == DC - 1))
            sumsq = ps_tile()
            for ko in range(DC):
                nc.tensor.matmul(sumsq, onesPP, xsq[:, ko, :],
                                 start=(ko == 0), stop=(ko == DC - 1))
            mu = mupool.tile([128, 512], f32, tag="mu")
            nc.vector.tensor_scalar_mul(mu, sums, 1.0 / D)
            t = lnpool.tile([128, 512], f32, tag="lntmp")
            nc.vector.tensor_mul(t, mu, sums)
            v = lnpool.tile([128, 512], f32, tag="lntmp")
            nc.vector.tensor_tensor(v, sumsq, t, OP.subtract)
            sd = lnpool.tile([128, 512], f32, tag="lntmp")
            nc.scalar.activation(sd, v, AF.Sqrt, bias=epsT, scale=1.0 / D)
            rstd = lnpool.tile([128, 512], f32, tag="lntmp")
            nc.vector.reciprocal_approx_fast(out=rstd, in_=sd)
            mu_b = mupool.tile([128, 512], bf16, tag="mub")
            nc.vector.tensor_copy(mu_b, mu)
            rstd_b = mupool.tile([128, 512], bf16, tag="rstdb")
            nc.vector.tensor_copy(rstd_b, rstd)
            return mu_b, rstd_b, xb

        def ln_apply(out_tile):
            """out_tile[:, ko, t] = (x - mu) * rstd  (affine folded into weights)"""
            for c in range(3):
                sl = slice(c * 512, (c + 1) * 512)
                mu_b, rstd_b, xb = ln_stats(sl)
                nc.vector.tensor_tensor(
                    out_tile[:, :, sl], xb,
                    mu_b[:, None, :].to_broadcast((128, DC, 512)), OP.subtract)
                nc.vector.tensor_tensor(
                    out_tile[:, :, sl], out_tile[:, :, sl],
                    rstd_b[:, None, :].to_broadcast((128, DC, 512)), OP.mult)

        def ln_final(sT, bT):
            """final LN with affine, normalized part in bf16, in-place on xT."""
            c1 = bigpool.tile([128, DC, TP], bf16, tag="yT")
            for c in range(3):
                sl = slice(c * 512, (c + 1) * 512)
                mu_b, rstd_b, xb = ln_stats(sl)
                nc.vector.tensor_tensor(
                    c1[:, :, sl], xb,
                    mu_b[:, None, :].to_broadcast((128, DC, 512)), OP.subtract)
                nc.vector.tensor_tensor(
                    c1[:, :, sl], c1[:, :, sl],
                    rstd_b[:, None, :].to_broadcast((128, DC, 512)), OP.mult)
                for ko in range(DC):
                    nc.vector.tensor_scalar(
                        out=xT[:, ko, sl], in0=c1[:, ko, sl],
                        scalar1=sT[:, ko:ko + 1], scalar2=bT[:, ko:ko + 1],
                        op0=OP.mult, op1=OP.add)

        def load_w512(dram_ap_chunked, cols):
            """load [128, DC-ish, cols] bf16 weight chunk"""
            n = cols.stop - cols.start
            kdim = dram_ap_chunked.shape[1]
            wt = wpool.tile([128, kdim, n], bf16, tag="w512")
            nc.sync.dma_start(wt, dram_ap_chunked[:, :, cols])
            return wt

        for l in range(n_layers):
            # ---------- LN1 (affine folded into wqkv/bqkv) ----------
            yT = bigpool.tile([128, DC, TP], bf16, tag="yT")
            ln_apply(yT)

            # ---------- QKV ----------
            wq_ch = dchunked(wqkv_d[l])
            bqkv = bpool.tile([128, 12], f32, tag="bqkv")
            nc.sync.dma_start(bqkv, bqkvT_d[l])

            # V: natural layout -> vone slots  (out tokens on partitions)
            # (V bias folded host-side into the wo residual bias)
            wv_a = load_w512(wq_ch, slice(1536, 2048))
            wv_b = load_w512(wq_ch, slice(2048, 2304))
            for tt in range(NT):
                for (wt, c0, cl, h0, hn) in ((wv_a, 0, 512, 0, 8), (wv_b, 512, 256, 8, 4)):
                    pv = ps_tile()[:, :cl]
                    for ko in range(DC):
                        nc.tensor.matmul(pv, yT[:, ko, tt * 128:(tt + 1) * 128],
                                         wt[:, ko, :], start=(ko == 0),
                                         stop=(ko == DC - 1))
                    vslots = vone[:, tt, :].rearrange("p (h s) -> p h s", s=128)
                    nc.vector.tensor_copy(
                        vslots[:, h0:h0 + hn, 0:64],
                        pv.rearrange("p (h s) -> p h s", s=64))

            # QK per head pair + attention
            wq_tiles = [load_w512(wq_ch, slice(512 * i, 512 * (i + 1)))
                        for i in range(3)]
            OT = bigpool.tile([128, DC, TP], bf16, tag="OT")
            for pair in range(6):
                qk = qkpool.tile([128, 2, TP], bf16, tag="qk")
                for i, m in enumerate((pair, 6 + pair)):
                    wt = wq_tiles[(m * 128) // 512]
                    for c in range(3):
                        ps = ps_tile()
                        coff = (m * 128) % 512
                        for ko in range(DC):
                            nc.tensor.matmul(ps, wt[:, ko, coff:coff + 128],
                                             yT[:, ko, c * 512:(c + 1) * 512],
                                             start=(ko == 0), stop=(ko == DC - 1))
                        nc.vector.tensor_scalar_add(qk[:, i, c * 512:(c + 1) * 512],
                                                    ps, bqkv[:, m:m + 1])
                # scores for both heads interleaved: the two 64-contraction
                # matmuls land on disjoint PE row-groups (tile_position
                # (0,0)/(64,0) auto-derived) and run concurrently.
                for kt in range(NT):
                    qs = 0 if kt == 0 else 128 * kt
                    off = ARENA_OFF[kt]
                    for g0 in range(qs, TP, 512):
                        g1 = min(g0 + 512, TP)
                        for e in range(2):
                            st = ps_tile()[:, 0:g1 - g0]
                            nc.tensor.matmul(
                                st, qk[64 * e:64 * e + 64, 1, kt * 128:(kt + 1) * 128],
                                qk[64 * e:64 * e + 64, 0, g0:g1],
                                start=True, stop=True)
                            pslice = ptarena2[:, e, off + g0 - qs:off + g1 - qs]
                            nc.scalar.activation(out=pslice, in_=st, func=AF.Exp,
                                                 bias=maskcol[:, kt:kt + 1], scale=0.125)
                    if 2 <= kt <= 10:
                        u = kt - 1
                        nc.vector.memset(
                            ptarena2[:, :, off + 1408 - qs:off + 1408 - qs + 8 * u], 0.0)
                    if kt == 11:
                        nc.vector.tensor_mul(
                            ptarena2[:, :, off:off + 128], ptarena2[:, :, off:off + 128],
                            rrm[:, None, :].to_broadcast((128, 2, 128)))
                for e in range(2):
                    h = 2 * pair + e
                    for qc in range(3):
                        c0, c1_ = qc * 512, (qc + 1) * 512
                        kts = [kt for kt in range(NT)
                               if (0 if kt == 0 else 128 * kt) < c1_]
                        ot = ps_ot.tile([128, 512], f32, tag="ot")
                        for i, kt in enumerate(kts):
                            qs = 0 if kt == 0 else 128 * kt
                            off = ARENA_OFF[kt]
                            lo = max(qs, c0)
                            nc.tensor.matmul(ot[:, lo - c0:512],
                                             vone[:, kt, 128 * h:128 * h + 128],
                                             ptarena2[:, e, off + lo - qs:off + c1_ - qs],
                                             start=(i == 0), stop=(i == len(kts) - 1),
                                             skip_group_check=(i > 0))
                        rec = recpool.tile([128, 1024], f32, tag="rec")
                        nc.vector.tensor_copy(rec[0:64, 512:1024], ot[64:128, :])
                        nc.vector.reciprocal_approx_fast(
                            out=rec[0:64, 0:512], in_=rec[0:64, 512:1024])
                        nc.vector.tensor_tensor(OT[64 * e:64 * e + 64, pair, c0:c1_],
                                                ot[0:64, :], rec[0:64, 0:512], OP.mult)

            # ---------- WO + residual (bias fused into residual add) ----------
            wo_ch = dchunked(wo_d[l])
            wo_a = load_w512(wo_ch, slice(0, 512))
            wo_b = load_w512(wo_ch, slice(512, 768))
            bocol = bpool.tile([128, DC], f32, tag="bocol")
            nc.sync.dma_start(bocol, bocolT_d[l])
            for c in range(3):
                for dc in range(DC):
                    wt, coff = (wo_a, dc * 128) if dc < 4 else (wo_b, (dc - 4) * 128)
                    ps = ps_tile()
                    for ko in range(DC):
                        nc.tensor.matmul(ps, wt[:, ko, coff:coff + 128],
                                         OT[:, ko, c * 512:(c + 1) * 512],
                                         start=(ko == 0), stop=(ko == DC - 1))
                    nc.vector.scalar_tensor_tensor(
                        out=xT[:, dc, c * 512:(c + 1) * 512],
                        in0=ps, scalar=bocol[:, dc:dc + 1],
                        in1=xT[:, dc, c * 512:(c + 1) * 512],
                        op0=OP.add, op1=OP.add)

            # ---------- LN2 (affine folded into w1/b1) ----------
            yT2 = bigpool.tile([128, DC, TP], bf16, tag="yT")
            ln_apply(yT2)

            # ---------- FFN (4 quarters of F) ----------
            w1_ch = dchunked(w1_d[l])
            w2_ch = w2_d[l].rearrange("(fo p) n -> p fo n", p=128)
            b1T = bpool.tile([128, 24], f32, tag="b1T")
            nc.sync.dma_start(b1T, b1T_d[l])
            b2col = bpool.tile([128, DC], f32, tag="b2col")
            nc.sync.dma_start(b2col, b2colT_d[l])
            for q4 in range(4):
                f0 = 768 * q4
                w1_a = load_w512(w1_ch, slice(f0, f0 + 512))
                w1_b = load_w512(w1_ch, slice(f0 + 512, f0 + 768))
                hT = bigpool.tile([128, DC, TP], bf16, tag="hT")
                for fm in range(6):
                    wt, coff = (w1_a, fm * 128) if fm < 4 else (w1_b, (fm - 4) * 128)
                    for c in range(3):
                        ps = ps_tile()
                        for ko in range(DC):
                            nc.tensor.matmul(ps, wt[:, ko, coff:coff + 128],
                                             yT2[:, ko, c * 512:(c + 1) * 512],
                                             start=(ko == 0), stop=(ko == DC - 1))
                        nc.scalar.activation(out=hT[:, fm, c * 512:(c + 1) * 512], in_=ps,
                                             func=AF.Gelu_apprx_tanh,
                                             bias=b1T[:, 6 * q4 + fm:6 * q4 + fm + 1])
                w2q = w2_ch[:, 6 * q4:6 * q4 + 6, :]
                w2_a = wpool.tile([128, 6, 512], bf16, tag="w512")
                nc.sync.dma_start(w2_a, w2q[:, :, 0:512])
                w2_b = wpool.tile([128, 6, 256], bf16, tag="w512")
                nc.sync.dma_start(w2_b, w2q[:, :, 512:768])
                for c in range(3):
                    for dc in range(DC):
                        wt, coff = (w2_a, dc * 128) if dc < 4 else (w2_b, (dc - 4) * 128)
                        ps = ps_tile()
                        for fo in range(6):
                            nc.tensor.matmul(ps, wt[:, fo, coff:coff + 128],
                                             hT[:, fo, c * 512:(c + 1) * 512],
                                             start=(fo == 0), stop=(fo == 5))
                        if q4 == 0:
                            nc.vector.scalar_tensor_tensor(
                                out=xT[:, dc, c * 512:(c + 1) * 512],
                                in0=ps, scalar=b2col[:, dc:dc + 1],
                                in1=xT[:, dc, c * 512:(c + 1) * 512],
                                op0=OP.add, op1=OP.add)
                        else:
                            nc.vector.tensor_tensor(
                                xT[:, dc, c * 512:(c + 1) * 512],
                                xT[:, dc, c * 512:(c + 1) * 512],
                                ps, OP.add)

        # ---------- final LN (in place) + store ----------
        ln_final(lnfs, lnfb)
        nc.sync.dma_start(dchunked(outT_d[:]), xT)

    nc.compile()
    return nc


# ---------------- host-side glue ----------------

def _prep_weights(inputs, n_layers):
    bf = ml_dtypes.bfloat16
    sl = slice(0, n_layers)

    def dT(a):  # [..., 768] -> [..., 128, 6] (d = ko*128 + p)
        return np.ascontiguousarray(np.swapaxes(a.reshape(*a.shape[:-1], DC, 128), -1, -2))

    ln1s = np.asarray(inputs["ln1_s"][sl], np.float32)
    ln1b = np.asarray(inputs["ln1_b"][sl], np.float32)
    ln2s = np.asarray(inputs["ln2_s"][sl], np.float32)
    ln2b = np.asarray(inputs["ln2_b"][sl], np.float32)

    # fold LN1 affine into wqkv/bqkv, LN2 affine into w1/b1
    wqkv = np.asarray(inputs["wqkv"][sl], np.float32)
    bqkv = np.asarray(inputs["bqkv"][sl], np.float32)
    wqkv_f = ln1s[:, :, None] * wqkv
    bqkv_f = bqkv + np.einsum('ld,lde->le', ln1b, wqkv)
    w1 = np.asarray(inputs["w1"][sl], np.float32)
    b1 = np.asarray(inputs["b1"][sl], np.float32)
    w1_f = ln2s[:, :, None] * w1
    b1_f = b1 + np.einsum('ld,lde->le', ln2b, w1)

    bqkvT = np.ascontiguousarray(
        np.swapaxes(bqkv_f[:, :1536].reshape(n_layers, 12, 128), 1, 2))
    bv = bqkv_f[:, 1536:]                        # V bias -> fold into wo bias
    b1T = np.ascontiguousarray(np.swapaxes(b1_f.reshape(n_layers, 24, 128), 1, 2))
    wo = np.asarray(inputs["wo"][sl], np.float32)
    bo_f = np.asarray(inputs["bo"][sl], np.float32) + np.einsum('ld,lde->le', bv, wo)
    return {
        "wqkv": np.ascontiguousarray(wqkv_f).astype(bf),
        "bqkvT": np.ascontiguousarray(bqkvT, np.float32),
        "wo": np.ascontiguousarray(wo).astype(bf),
        "bocolT": np.ascontiguousarray(dT(bo_f)),
        "w1": np.ascontiguousarray(w1_f).astype(bf),
        "b1T": np.ascontiguousarray(b1T, np.float32),
        "w2": np.ascontiguousarray(inputs["w2"][sl]).astype(bf),
        "b2colT": np.ascontiguousarray(dT(np.asarray(inputs["b2"][sl], np.float32))),
        "lnfsT": np.ascontiguousarray(dT(np.asarray(inputs["lnf_s"], np.float32))),
        "lnfbT": np.ascontiguousarray(dT(np.asarray(inputs["lnf_b"], np.float32))),
    }


def _rrmask():
    m = np.zeros((128, 128), np.float32)
    for k in range(80):
        m[k, (k // 8) * 8:] = 1.0
    return m.astype(ml_dtypes.bfloat16)


def _maskcol(prefix_mask, obs_mask, readout_mask):
    """[128, 12] additive exp-bias per (key partition, key tile)."""
    m = np.full((128, NT), NEG, np.float32)
    m[:PFX, 0] = np.where(prefix_mask, 0.0, NEG)
    for t in range(HOR):
        m[:, 1 + t] = np.where(obs_mask[t], 0.0, NEG)
    ro = np.asarray(readout_mask).reshape(-1)
    m[:80, 11] = np.where(ro, 0.0, NEG)
    return m


def _assemble_xT(prefix, obs, readout):
    """(16,768),(10,128,768),(10,8,768) -> transposed padded (768,1536) f32"""
    x = np.zeros((TP, D), np.float32)
    x[:PFX] = prefix
    x[128:128 + HOR * 128] = obs.reshape(HOR * 128, D)
    x[1408:1408 + HOR * NR] = readout.reshape(HOR * NR, D)
    return np.ascontiguousarray(x.T)


def _gather_out(outT):
    """(768,1536) -> (1376, 768) in original token order"""
    xo = outT.T
    out = np.empty((T, D), np.float32)
    out[:PFX] = xo[:PFX]
    for t in range(HOR):
        out[PFX + TPS * t:PFX + TPS * t + NO] = xo[128 * (1 + t):128 * (2 + t)]
        out[PFX + TPS * t + NO:PFX + TPS * (t + 1)] = xo[1408 + NR * t:1408 + NR * (t + 1)]
    return out


def run(inputs, n_layers=L, trace=False, tmpdir=None):
    from concourse.bass_utils import run_bass_kernel_spmd

    key = ("nc", n_layers)
    if key not in _CACHE:
        _CACHE[key] = _build_nc(n_layers)
    nc = _CACHE[key]

    wmap = _prep_weights(inputs, n_layers)
    rr = _rrmask()
    pm = np.asarray(inputs["prefix_mask"], bool)
    om = np.asarray(inputs["obs_mask"], bool)
    rm = np.asarray(inputs["readout_mask"], bool)
    pt = np.asarray(inputs["prefix_tokens"], np.float32)
    ot = np.asarray(inputs["obs_tokens"], np.float32)
    rt = np.asarray(inputs["readout_tokens"], np.float32)

    in_maps = []
    for b in range(B):
        m = dict(wmap)
        m["xT"] = _assemble_xT(pt[b], ot[b], rt[b])
        m["maskcol"] = _maskcol(pm[b], om[b], rm[b])
        m["rrmask"] = rr
        in_maps.append(m)

    res = run_bass_kernel_spmd(nc, in_maps, list(range(B)), trace=trace, tmpdir=tmpdir)
    out = np.stack([_gather_out(res.results[b]["outT"]) for b in range(B)])
    return out, res


def kernel(**inputs):
    out, _ = run(inputs, L)
    return out


# revision 31
# speedup vs baseline: 1.2932x; 1.0025x over previous
"""Trainium2 Bass kernel for nn_BlockTransformer (Octo-style block-sparse transformer).

Strategy: data-parallel over batch (B=8 -> 1 element per NeuronCore), weights
replicated. Residual stream kept transposed (D on partitions) to avoid all
on-device transposes. Tokens reordered to [prefix|pad, obs t0..t9 (128-aligned),
readouts|pad] = 1536 padded tokens so the block-sparse attention mask becomes
128-aligned; per-key mask folded into the softmax-exp bias (per-partition),
readout causality handled by small memsets + one static 0/1 multiplier tile.
Matmuls in bf16 with fp32 PSUM accumulation; residual stream fp32.

v2: LN affine folded into wqkv/w1 host-side; LN normalize in bf16 (2x DVE);
softmax reciprocal via reciprocal_approx_fast; biases for wo/w2 fused into the
residual add (scalar_tensor_tensor); 512-wide PSUM tiles with deep rotation.
"""
import sys
sys.path.insert(0, "/opt/trn_rl_repo")

import numpy as np
import ml_dtypes

B, HOR, PFX, NO, NR = 8, 10, 16, 128, 8
D, NH, HD, F, L = 768, 12, 64, 3072, 12
TPS = NO + NR
T = PFX + HOR * TPS          # 1376
TP = 1536                    # padded tokens (12 tiles of 128)
TQ = 1488                    # live tokens (prefix tile + 10 obs tiles + 80 readouts)
NT = TP // 128               # 12 token tiles
DC = D // 128                # 6 d-chunks
CHUNKS = [(0, 512), (512, 512), (1024, 464)]   # token chunks (c2 trimmed)
EPS = 1e-6
NEG = -30000.0

_CACHE = {}


def _build_nc(n_layers):
    from concourse import bacc
    import concourse.bass as bass
    import concourse.mybir as mybir
    import concourse.tile as tile
    from contextlib import ExitStack

    bf16, f32, f8 = mybir.dt.bfloat16, mybir.dt.float32, mybir.dt.float8e4
    AF = mybir.ActivationFunctionType
    OP = mybir.AluOpType

    nc = bacc.Bacc("TRN2", num_devices=8)

    xT_d = nc.dram_tensor("xT", [D, TP], f32, kind="ExternalInput")
    wqkv_d = nc.dram_tensor("wqkv", [n_layers, D, 3 * D], bf16, kind="ExternalInput")
    bqkvT_d = nc.dram_tensor("bqkvT", [n_layers, 128, 12], f32, kind="ExternalInput")
    wo_d = nc.dram_tensor("wo", [n_layers, D, D], bf16, kind="ExternalInput")
    bocolT_d = nc.dram_tensor("bocolT", [n_layers, 128, DC], f32, kind="ExternalInput")
    w1_d = nc.dram_tensor("w1", [n_layers, D, F], bf16, kind="ExternalInput")
    b1T_d = nc.dram_tensor("b1T", [n_layers, 128, 24], f32, kind="ExternalInput")
    w2_d = nc.dram_tensor("w2", [n_layers, F, D], bf16, kind="ExternalInput")
    b2colT_d = nc.dram_tensor("b2colT", [n_layers, 128, DC], f32, kind="ExternalInput")
    lnfsT_d = nc.dram_tensor("lnfsT", [128, DC], f32, kind="ExternalInput")
    lnfbT_d = nc.dram_tensor("lnfbT", [128, DC], f32, kind="ExternalInput")
    maskcol_d = nc.dram_tensor("maskcol", [128, NT], f32, kind="ExternalInput")
    rrmask_d = nc.dram_tensor("rrmask", [128, 128], bf16, kind="ExternalInput")
    outT_d = nc.dram_tensor("outT", [D, TP], f32, kind="ExternalOutput")

    def dchunked(ap):  # [D, N] dram AP -> [128, DC-chunks, N]
        return ap.rearrange("(ko p) n -> p ko n", p=128)

    with tile.TileContext(nc) as tc, ExitStack() as ctx:
        const = ctx.enter_context(tc.tile_pool(name="const", bufs=1))
        persist = ctx.enter_context(tc.tile_pool(name="persist", bufs=1))
        wpool = ctx.enter_context(tc.tile_pool(name="wpool", bufs=5))
        bpool = ctx.enter_context(tc.tile_pool(name="bpool", bufs=2))
        rowpool = ctx.enter_context(tc.tile_pool(name="rowpool", bufs=1))
        bigpool = ctx.enter_context(tc.tile_pool(name="bigpool", bufs=1))
        qkpool = ctx.enter_context(tc.tile_pool(name="qkpool", bufs=2))
        lnpool = ctx.enter_context(tc.tile_pool(name="lnpool", bufs=2))
        mupool = ctx.enter_context(tc.tile_pool(name="mupool", bufs=2))
        lnbig = ctx.enter_context(tc.tile_pool(name="lnbig", bufs=1))
        recpool = ctx.enter_context(tc.tile_pool(name="recpool", bufs=1))
        ps_g = ctx.enter_context(tc.tile_pool(name="ps_g", bufs=6, space="PSUM"))
        ps_ot = ctx.enter_context(tc.tile_pool(name="ps_ot", bufs=2, space="PSUM"))

        def ps_tile():
            tmm = ps_g.tile([128, 512], f32, tag="g")
            return tmm

        # ---- persistent state ----
        xT = persist.tile([128, DC, TP], f32)         # residual stream (transposed)
        nc.sync.dma_start(xT, dchunked(xT_d[:]))
        vone = persist.tile([128, NT, NH * 128], f8)  # per head: [V_h | ones]
        nc.vector.memset(vone, 1.0)
        ARENA_OFF = []
        _o = 0
        for kt in range(NT):
            ARENA_OFF.append(_o)
            _o += TQ - (0 if kt == 0 else 128 * kt)
        # two fp8 exp(score) arenas -> both heads of a pair live concurrently
        ptarena2 = persist.tile([128, 2, _o], f8, tag="ptarena2")

        # ---- constants ----
        maskcol = const.tile([128, NT], f32)
        nc.sync.dma_start(maskcol, maskcol_d[:])
        rrm = const.tile([128, 128], bf16)
        nc.sync.dma_start(rrm, rrmask_d[:])
        onesPP = const.tile([128, 128], bf16)
        nc.vector.memset(onesPP, 1.0)
        onerow = const.tile([1, 512], bf16)
        nc.vector.memset(onerow, 1.0)
        epsT = const.tile([128, 1], f32)
        nc.vector.memset(epsT, EPS)
        lnfs = const.tile([128, DC], f32)
        nc.sync.dma_start(lnfs, lnfsT_d[:])
        lnfb = const.tile([128, DC], f32)
        nc.sync.dma_start(lnfb, lnfbT_d[:])

        def ln_stats(c0, n):
            """returns (mu_b, rstd_b, xb) for token chunk [c0, c0+n)."""
            sl = slice(c0, c0 + n)
            xb = lnbig.tile([128, DC, 512], bf16, tag="xb")
            nc.gpsimd.tensor_copy(xb[:, :, :n], xT[:, :, sl])
            xsq = lnbig.tile([128, DC, 512], bf16, tag="xsq")
            nc.scalar.activation(xsq[:, :, :n], xT[:, :, sl], AF.Square)
            sums = ps_tile()[:, :n]
            for ko in range(DC):
                nc.tensor.matmul(sums, onesPP, xb[:, ko, :n],
                                 start=(ko == 0), stop=(ko == DC - 1))
            sumsq = ps_tile()[:, :n]
            for ko in range(DC):
                nc.tensor.matmul(sumsq, onesPP, xsq[:, ko, :n],
                                 start=(ko == 0), stop=(ko == DC - 1))
            mu_t = mupool.tile([128, 512], f32, tag="mu")
            mu = mu_t[:, :n]
            nc.vector.tensor_scalar_mul(mu, sums, 1.0 / D)
            t_t = lnpool.tile([128, 512], f32, tag="lntmp")
            t = t_t[:, :n]
            nc.vector.tensor_mul(t, mu, sums)
            v_t = lnpool.tile([128, 512], f32, tag="lntmp")
            v = v_t[:, :n]
            nc.vector.tensor_tensor(v, sumsq, t, OP.subtract)
            sd_t = lnpool.tile([128, 512], f32, tag="lntmp")
            sd = sd_t[:, :n]
            nc.scalar.activation(sd, v, AF.Sqrt, bias=epsT, scale=1.0 / D)
            rstd_t = lnpool.tile([128, 512], f32, tag="lntmp")
            rstd = rstd_t[:, :n]
            nc.vector.reciprocal_approx_fast(out=rstd, in_=sd)
            mub_t = mupool.tile([128, 512], bf16, tag="mub")
            mu_b = mub_t[:, :n]
            nc.vector.tensor_copy(mu_b, mu)
            rstdb_t = mupool.tile([128, 512], bf16, tag="rstdb")
            rstd_b = rstdb_t[:, :n]
            nc.vector.tensor_copy(rstd_b, rstd)
            return mu_b, rstd_b, xb

        def ln_apply(out_tile):
            """out_tile[:, ko, t] = (x - mu) * rstd  (affine folded into weights)"""
            nc.vector.memset(out_tile[:, :, TQ:TP], 0.0)   # keep dead tokens finite
            for c0, n in CHUNKS:
                sl = slice(c0, c0 + n)
                mu_b, rstd_b, xb = ln_stats(c0, n)
                nc.vector.tensor_tensor(
                    out_tile[:, :, sl], xb[:, :, :n],
                    mu_b[:, None, :].to_broadcast((128, DC, n)), OP.subtract)
                nc.vector.tensor_tensor(
                    out_tile[:, :, sl], out_tile[:, :, sl],
                    rstd_b[:, None, :].to_broadcast((128, DC, n)), OP.mult)

        def ln_final(sT, bT):
            """final LN with affine, normalized part in bf16, in-place on xT."""
            c1 = bigpool.tile([128, DC, TP], bf16, tag="yT")
            for c0, n in CHUNKS:
                sl = slice(c0, c0 + n)
                mu_b, rstd_b, xb = ln_stats(c0, n)
                nc.vector.tensor_tensor(
                    c1[:, :, sl], xb[:, :, :n],
                    mu_b[:, None, :].to_broadcast((128, DC, n)), OP.subtract)
                nc.vector.tensor_tensor(
                    c1[:, :, sl], c1[:, :, sl],
                    rstd_b[:, None, :].to_broadcast((128, DC, n)), OP.mult)
                for ko in range(DC):
                    nc.vector.tensor_scalar(
                        out=xT[:, ko, sl], in0=c1[:, ko, sl],
                        scalar1=sT[:, ko:ko + 1], scalar2=bT[:, ko:ko + 1],
                        op0=OP.mult, op1=OP.add)

        def load_w512(dram_ap_chunked, cols):
            """load [128, DC-ish, cols] bf16 weight chunk"""
            n = cols.stop - cols.start
            kdim = dram_ap_chunked.shape[1]
            wt = wpool.tile([128, kdim, n], bf16, tag="w512")
            nc.sync.dma_start(wt, dram_ap_chunked[:, :, cols])
            return wt

        for l in range(n_layers):
            # ---------- LN1 (affine folded into wqkv/bqkv) ----------
            yT = bigpool.tile([128, DC, TP], bf16, tag="yT")
            ln_apply(yT)

            # ---------- QKV ----------
            wq_ch = dchunked(wqkv_d[l])
            bqkv = bpool.tile([128, 12], f32, tag="bqkv")
            nc.sync.dma_start(bqkv, bqkvT_d[l])

            # V: natural layout -> vone slots  (out tokens on partitions)
            # (V bias folded host-side into the wo residual bias)
            wv_a = load_w512(wq_ch, slice(1536, 2048))
            wv_b = load_w512(wq_ch, slice(2048, 2304))
            for tt in range(NT):
                for (wt, c0, cl, h0, hn) in ((wv_a, 0, 512, 0, 8), (wv_b, 512, 256, 8, 4)):
                    pv = ps_tile()[:, :cl]
                    for ko in range(DC):
                        nc.tensor.matmul(pv, yT[:, ko, tt * 128:(tt + 1) * 128],
                                         wt[:, ko, :], start=(ko == 0),
                                         stop=(ko == DC - 1))
                    vslots = vone[:, tt, :].rearrange("p (h s) -> p h s", s=128)
                    nc.vector.tensor_copy(
                        vslots[:, h0:h0 + hn, 0:64],
                        pv.rearrange("p (h s) -> p h s", s=64))

            # QK per head pair + attention
            wq_tiles = [load_w512(wq_ch, slice(512 * i, 512 * (i + 1)))
                        for i in range(3)]
            OT = bigpool.tile([128, DC, TP], bf16, tag="OT")
            for pair in range(6):
                qk = qkpool.tile([128, 2, TP], bf16, tag="qk")
                nc.vector.memset(qk[:, :, TQ:TP], 0.0)     # dead-token K stays finite
                for i, m in enumerate((pair, 6 + pair)):
                    wt = wq_tiles[(m * 128) // 512]
                    for c0, n in CHUNKS:
                        ps = ps_tile()[:, :n]
                        coff = (m * 128) % 512
                        for ko in range(DC):
                            nc.tensor.matmul(ps, wt[:, ko, coff:coff + 128],
                                             yT[:, ko, c0:c0 + n],
                                             start=(ko == 0), stop=(ko == DC - 1))
                        nc.vector.tensor_scalar_add(qk[:, i, c0:c0 + n],
                                                    ps, bqkv[:, m:m + 1])
                # scores for both heads interleaved: the two 64-contraction
                # matmuls land on disjoint PE row-groups (tile_position
                # (0,0)/(64,0) auto-derived) and run concurrently.
                for kt in range(NT):
                    qs = 0 if kt == 0 else 128 * kt
                    off = ARENA_OFF[kt]
                    for g0 in range(qs, TQ, 512):
                        g1 = min(g0 + 512, TQ)
                        for e in range(2):
                            st = ps_tile()[:, 0:g1 - g0]
                            nc.tensor.matmul(
                                st, qk[64 * e:64 * e + 64, 1, kt * 128:(kt + 1) * 128],
                                qk[64 * e:64 * e + 64, 0, g0:g1],
                                start=True, stop=True)
                            pslice = ptarena2[:, e, off + g0 - qs:off + g1 - qs]
                            nc.scalar.activation(out=pslice, in_=st, func=AF.Exp,
                                                 bias=maskcol[:, kt:kt + 1], scale=0.125)
                    if 2 <= kt <= 10:
                        u = kt - 1
                        nc.vector.memset(
                            ptarena2[:, :, off + 1408 - qs:off + 1408 - qs + 8 * u], 0.0)
                    if kt == 11:
                        nc.vector.tensor_mul(
                            ptarena2[:, :, off:off + 80], ptarena2[:, :, off:off + 80],
                            rrm[:, None, 0:80].to_broadcast((128, 2, 80)))
                for e in range(2):
                    h = 2 * pair + e
                    for c0, n in CHUNKS:
                        c1_ = c0 + n
                        kts = [kt for kt in range(NT)
                               if (0 if kt == 0 else 128 * kt) < c1_]
                        ot = ps_ot.tile([128, 512], f32, tag="ot")
                        for i, kt in enumerate(kts):
                            qs = 0 if kt == 0 else 128 * kt
                            off = ARENA_OFF[kt]
                            lo = max(qs, c0)
                            nc.tensor.matmul(ot[:, lo - c0:n],
                                             vone[:, kt, 128 * h:128 * h + 128],
                                             ptarena2[:, e, off + lo - qs:off + c1_ - qs],
                                             start=(i == 0), stop=(i == len(kts) - 1),
                                             skip_group_check=(i > 0))
                        rec = recpool.tile([128, 1024], f32, tag="rec")
                        nc.vector.tensor_copy(rec[0:64, 512:512 + n], ot[64:128, :n])
                        nc.vector.reciprocal_approx_fast(
                            out=rec[0:64, 0:n], in_=rec[0:64, 512:512 + n])
                        nc.vector.tensor_tensor(OT[64 * e:64 * e + 64, pair, c0:c1_],
                                                ot[0:64, :n], rec[0:64, 0:n], OP.mult)

            # ---------- WO + residual (bias fused into residual add) ----------
            wo_ch = dchunked(wo_d[l])
            wo_a = load_w512(wo_ch, slice(0, 512))
            wo_b = load_w512(wo_ch, slice(512, 768))
            bocol = bpool.tile([128, DC], f32, tag="bocol")
            nc.sync.dma_start(bocol, bocolT_d[l])
            for c0, n in CHUNKS:
                for dc in range(DC):
                    wt, coff = (wo_a, dc * 128) if dc < 4 else (wo_b, (dc - 4) * 128)
                    ps = ps_tile()[:, :n]
                    for ko in range(DC):
                        nc.tensor.matmul(ps, wt[:, ko, coff:coff + 128],
                                         OT[:, ko, c0:c0 + n],
                                         start=(ko == 0), stop=(ko == DC - 1))
                    nc.vector.scalar_tensor_tensor(
                        out=xT[:, dc, c0:c0 + n],
                        in0=ps, scalar=bocol[:, dc:dc + 1],
                        in1=xT[:, dc, c0:c0 + n],
                        op0=OP.add, op1=OP.add)

            # ---------- LN2 (affine folded into w1/b1) ----------
            yT2 = bigpool.tile([128, DC, TP], bf16, tag="yT")
            ln_apply(yT2)

            # ---------- FFN (4 quarters of F) ----------
            w1_ch = dchunked(w1_d[l])
            w2_ch = w2_d[l].rearrange("(fo p) n -> p fo n", p=128)
            b1T = bpool.tile([128, 24], f32, tag="b1T")
            nc.sync.dma_start(b1T, b1T_d[l])
            b2col = bpool.tile([128, DC], f32, tag="b2col")
            nc.sync.dma_start(b2col, b2colT_d[l])
            for q4 in range(4):
                f0 = 768 * q4
                w1_a = load_w512(w1_ch, slice(f0, f0 + 512))
                w1_b = load_w512(w1_ch, slice(f0 + 512, f0 + 768))
                hT = bigpool.tile([128, DC, TP], bf16, tag="hT")
                for fm in range(6):
                    wt, coff = (w1_a, fm * 128) if fm < 4 else (w1_b, (fm - 4) * 128)
                    for c0, n in CHUNKS:
                        ps = ps_tile()[:, :n]
                        for ko in range(DC):
                            nc.tensor.matmul(ps, wt[:, ko, coff:coff + 128],
                                             yT2[:, ko, c0:c0 + n],
                                             start=(ko == 0), stop=(ko == DC - 1))
                        nc.scalar.activation(out=hT[:, fm, c0:c0 + n], in_=ps,
                                             func=AF.Gelu_apprx_tanh,
                                             bias=b1T[:, 6 * q4 + fm:6 * q4 + fm + 1])
                w2q = w2_ch[:, 6 * q4:6 * q4 + 6, :]
                w2_a = wpool.tile([128, 6, 512], bf16, tag="w512")
                nc.sync.dma_start(w2_a, w2q[:, :, 0:512])
                w2_b = wpool.tile([128, 6, 256], bf16, tag="w512")
                nc.sync.dma_start(w2_b, w2q[:, :, 512:768])
                for c0, n in CHUNKS:
                    for dc in range(DC):
                        wt, coff = (w2_a, dc * 128) if dc < 4 else (w2_b, (dc - 4) * 128)
                        ps = ps_tile()[:, :n]
                        for fo in range(6):
                            nc.tensor.matmul(ps, wt[:, fo, coff:coff + 128],
                                             hT[:, fo, c0:c0 + n],
                                             start=(fo == 0), stop=(fo == 5))
                        if q4 == 0:
                            nc.vector.scalar_tensor_tensor(
                                out=xT[:, dc, c0:c0 + n],
                                in0=ps, scalar=b2col[:, dc:dc + 1],
                                in1=xT[:, dc, c0:c0 + n],
                                op0=OP.add, op1=OP.add)
                        else:
                            nc.vector.tensor_tensor(
                                xT[:, dc, c0:c0 + n],
                                xT[:, dc, c0:c0 + n],
                                ps, OP.add)

        # ---------- final LN (in place) + store ----------
        ln_final(lnfs, lnfb)
        nc.sync.dma_start(dchunked(outT_d[:]), xT)

    nc.compile()
    return nc


# ---------------- host-side glue ----------------

def _prep_weights(inputs, n_layers):
    bf = ml_dtypes.bfloat16
    sl = slice(0, n_layers)

    def dT(a):  # [..., 768] -> [..., 128, 6] (d = ko*128 + p)
        return np.ascontiguousarray(np.swapaxes(a.reshape(*a.shape[:-1], DC, 128), -1, -2))

    ln1s = np.asarray(inputs["ln1_s"][sl], np.float32)
    ln1b = np.asarray(inputs["ln1_b"][sl], np.float32)
    ln2s = np.asarray(inputs["ln2_s"][sl], np.float32)
    ln2b = np.asarray(inputs["ln2_b"][sl], np.float32)

    # fold LN1 affine into wqkv/bqkv, LN2 affine into w1/b1
    wqkv = np.asarray(inputs["wqkv"][sl], np.float32)
    bqkv = np.asarray(inputs["bqkv"][sl], np.float32)
    wqkv_f = ln1s[:, :, None] * wqkv
    bqkv_f = bqkv + np.einsum('ld,lde->le', ln1b, wqkv)
    w1 = np.asarray(inputs["w1"][sl], np.float32)
    b1 = np.asarray(inputs["b1"][sl], np.float32)
    w1_f = ln2s[:, :, None] * w1
    b1_f = b1 + np.einsum('ld,lde->le', ln2b, w1)

    bqkvT = np.ascontiguousarray(
        np.swapaxes(bqkv_f[:, :1536].reshape(n_layers, 12, 128), 1, 2))
    bv = bqkv_f[:, 1536:]                        # V bias -> fold into wo bias
    b1T = np.ascontiguousarray(np.swapaxes(b1_f.reshape(n_layers, 24, 128), 1, 2))
    wo = np.asarray(inputs["wo"][sl], np.float32)
    bo_f = np.asarray(inputs["bo"][sl], np.float32) + np.einsum('ld,lde->le', bv, wo)
    return {
        "wqkv": np.ascontiguousarray(wqkv_f).astype(bf),
        "bqkvT": np.ascontiguousarray(bqkvT, np.float32),
        "wo": np.ascontiguousarray(wo).astype(bf),
        "bocolT": np.ascontiguousarray(dT(bo_f)),
        "w1": np.ascontiguousarray(w1_f).astype(bf),
        "b1T": np.ascontiguousarray(b1T, np.float32),
        "w2": np.ascontiguousarray(inputs["w2"][sl]).astype(bf),
        "b2colT": np.ascontiguousarray(dT(np.asarray(inputs["b2"][sl], np.float32))),
        "lnfsT": np.ascontiguousarray(dT(np.asarray(inputs["lnf_s"], np.float32))),
        "lnfbT": np.ascontiguousarray(dT(np.asarray(inputs["lnf_b"], np.float32))),
    }


def _rrmask():
    m = np.zeros((128, 128), np.float32)
    for k in range(80):
        m[k, (k // 8) * 8:] = 1.0
    return m.astype(ml_dtypes.bfloat16)


def _maskcol(prefix_mask, obs_mask, readout_mask):
    """[128, 12] additive exp-bias per (key partition, key tile)."""
    m = np.full((128, NT), NEG, np.float32)
    m[:PFX, 0] = np.where(prefix_mask, 0.0, NEG)
    for t in range(HOR):
        m[:, 1 + t] = np.where(obs_mask[t], 0.0, NEG)
    ro = np.asarray(readout_mask).reshape(-1)
    m[:80, 11] = np.where(ro, 0.0, NEG)
    return m


def _assemble_xT(prefix, obs, readout):
    """(16,768),(10,128,768),(10,8,768) -> transposed padded (768,1536) f32"""
    x = np.zeros((TP, D), np.float32)
    x[:PFX] = prefix
    x[128:128 + HOR * 128] = obs.reshape(HOR * 128, D)
    x[1408:1408 + HOR * NR] = readout.reshape(HOR * NR, D)
    return np.ascontiguousarray(x.T)


def _gather_out(outT):
    """(768,1536) -> (1376, 768) in original token order"""
    xo = outT.T
    out = np.empty((T, D), np.float32)
    out[:PFX] = xo[:PFX]
    for t in range(HOR):
        out[PFX + TPS * t:PFX + TPS * t + NO] = xo[128 * (1 + t):128 * (2 + t)]
        out[PFX + TPS * t + NO:PFX + TPS * (t + 1)] = xo[1408 + NR * t:1408 + NR * (t + 1)]
    return out


def run(inputs, n_layers=L, trace=False, tmpdir=None):
    from concourse.bass_utils import run_bass_kernel_spmd

    key = ("nc", n_layers)
    if key not in _CACHE:
        _CACHE[key] = _build_nc(n_layers)
    nc = _CACHE[key]

    wmap = _prep_weights(inputs, n_layers)
    rr = _rrmask()
    pm = np.asarray(inputs["prefix_mask"], bool)
    om = np.asarray(inputs["obs_mask"], bool)
    rm = np.asarray(inputs["readout_mask"], bool)
    pt = np.asarray(inputs["prefix_tokens"], np.float32)
    ot = np.asarray(inputs["obs_tokens"], np.float32)
    rt = np.asarray(inputs["readout_tokens"], np.float32)

    in_maps = []
    for b in range(B):
        m = dict(wmap)
        m["xT"] = _assemble_xT(pt[b], ot[b], rt[b])
        m["maskcol"] = _maskcol(pm[b], om[b], rm[b])
        m["rrmask"] = rr
        in_maps.append(m)

    res = run_bass_kernel_spmd(nc, in_maps, list(range(B)), trace=trace, tmpdir=tmpdir)
    out = np.stack([_gather_out(res.results[b]["outT"]) for b in range(B)])
    return out, res


def kernel(**inputs):
    out, _ = run(inputs, L)
    return out
